# revision 1
# baseline (speedup 1.0000x reference)
"""Causal self-attention with RoPE on 8 Trainium2 NeuronCores.

Sharding: batch (4) x head-group (2 groups of 8 heads) -> 8 cores.
Each core computes, for its (batch b, head group g):
  qkv projection (fp32r matmuls, full PE rate, ~1.5e-4 rel err),
  RoPE (DVE + DMA partition shift),
  causal attention (scores fp32r K=64 row-concurrent pairs, softmax exp on
  ACT with folded 1/8 scale, probs/V in fp16, exact-sum normalization),
  output projection partial (fp32r).
Host sums the two head-group partials per batch.
"""
import sys

sys.path.insert(0, "/opt/trn_rl_repo")

import numpy as np

import concourse.bass as bass  # noqa: F401
import concourse.mybir as mybir
import concourse.tile as tile
from concourse import bacc
from concourse.bass_utils import run_bass_kernel_spmd

dt = mybir.dt
F32, F32R, F16 = dt.float32, dt.float32r, dt.float16
ALU = mybir.AluOpType
EXP = mybir.ActivationFunctionType.Exp

ROPE_BASE = 10000.0


def build_core_program(S=2048, D=1024, HL=8, hd=64):
    """Bass program for one core. See module docstring; v2 schedule:
    v-proj | qk pairs 0,1 | attn 0 | attn 1 | qk pairs 2,3 | attn 2 |
    attn 3 | out-proj, so softmax (ACT) overlaps projection matmuls (PE).
    """
    assert hd == 64
    NP = HL // 2
    DT = D // 128
    SC = S // 512
    ST = S // 128
    NJT = 2 * NP
    IC = S // 512
    scale = hd ** -0.5

    nc = bacc.Bacc("TRN2", target_bir_lowering=False, debug=False)
    xT_d = nc.dram_tensor("xT", [D, S], F16, kind="ExternalInput").ap()
    Wqk_d = nc.dram_tensor("Wqk", [D, NJT * 128], F16, kind="ExternalInput").ap()
    Wv_d = nc.dram_tensor("Wv", [D, HL * hd], F16, kind="ExternalInput").ap()
    Wout_d = nc.dram_tensor("Wout", [HL * hd, D], F16, kind="ExternalInput").ap()
    cos_d = nc.dram_tensor("cosT", [128, S], F16, kind="ExternalInput").ap()
    s2_d = nc.dram_tensor("S2T", [128, S], F16, kind="ExternalInput").ap()
    tri_d = nc.dram_tensor("tri", [128, 128], F16, kind="ExternalInput").ap()
    y_d = nc.dram_tensor("y", [S, D], F32, kind="ExternalOutput").ap()

    with tile.TileContext(nc) as tc:
        with tc.tile_pool(name="persist", bufs=1) as pp, \
             tc.tile_pool(name="w16", bufs=2) as w16, \
             tc.tile_pool(name="x2", bufs=8) as x2p, \
             tc.tile_pool(name="s512", bufs=4) as s512, \
             tc.tile_pool(name="expp", bufs=6) as expp, \
             tc.tile_pool(name="normp", bufs=2) as normp, \
             tc.tile_pool(name="ps1", bufs=2, space="PSUM") as gpps, \
             tc.tile_pool(name="sps", bufs=3, space="PSUM") as sps:

            qkT = [pp.tile([128, S], F32R, tag=f"qkT{j}", name=f"qkT{j}")
                   for j in range(NJT)]
            v_sb = pp.tile([128, ST, HL, 66], F16, tag="v_sb")
            outT = [pp.tile([128, S], F16, tag=f"outT{p}", name=f"outT{p}")
                    for p in range(NP)]
            cosT = pp.tile([128, S], F16, tag="cosT")
            s2T = pp.tile([128, S], F16, tag="s2T")
            tri = pp.tile([128, 128], F16, tag="tri")
            nc.sync.dma_start(out=cosT[:], in_=cos_d[:])
            nc.sync.dma_start(out=s2T[:], in_=s2_d[:])
            nc.sync.dma_start(out=tri[:], in_=tri_d[:])
            nc.vector.memset(v_sb[:, :, :, 64:65], 1.0)

            def load_x_chunk(sc):
                xq = [x2p.tile([128, 2, 512], F16, tag="x2", name="xq")
                      for _ in range(DT // 2)]
                for ddt in range(DT):
                    nc.sync.dma_start(
                        out=xq[ddt // 2][:, ddt % 2, :],
                        in_=xT_d[ddt * 128:(ddt + 1) * 128,
                                 sc * 512:(sc + 1) * 512])
                return xq

            # ---------------- V projection ----------------
            wv = w16.tile([128, DT, 512], F16, tag="w16", name="wv")
            for ddt in range(DT):
                nc.sync.dma_start(
                    out=wv[:, ddt, :], in_=Wv_d[ddt * 128:(ddt + 1) * 128, :])
            for sc in range(SC):
                xq = load_x_chunk(sc)
                for stl in range(4):
                    st = sc * 4 + stl
                    vps = gpps.tile([128, 512], F32, tag="ps1", name="vps")
                    for ddt in range(DT):
                        nc.tensor.matmul(
                            vps[:],
                            xq[ddt // 2][:, ddt % 2, stl * 128:(stl + 1) * 128],
                            wv[:, ddt, :], start=(ddt == 0), stop=(ddt == DT - 1))
                    nc.vector.tensor_copy(
                        v_sb[:, st, :, 0:64],
                        vps[:].rearrange("p (h c) -> p h c", h=HL))

            # ---------------- QK projection + RoPE (one j-half) ----------------
            def qk_load_w(jh):
                wqk = w16.tile([128, DT, 512], F16, tag="w16", name="wqk")
                for ddt in range(DT):
                    nc.sync.dma_start(
                        out=wqk[:, ddt, :],
                        in_=Wqk_d[ddt * 128:(ddt + 1) * 128,
                                  jh * 512:(jh + 1) * 512])
                return wqk

            def qk_group(jh, wqk, jl, sc, xq):
                if True:
                    if True:
                        jt = jh * 4 + jl
                        qkps = gpps.tile([128, 512], F32, tag="ps1", name="qkps")
                        for ddt in range(DT):
                            nc.tensor.matmul(
                                qkps[:], wqk[:, ddt, jl * 128:(jl + 1) * 128],
                                xq[ddt // 2][:, ddt % 2, :],
                                start=(ddt == 0), stop=(ddt == DT - 1))
                        ss = slice(sc * 512, (sc + 1) * 512)
                        qraw = s512.tile([128, 512], F32, tag="qraw", name="qraw")
                        nc.vector.tensor_copy(qraw[:], qkps[:])
                        rot = s512.tile([128, 512], F32, tag="rot", name="rot")
                        for b0 in range(4):
                            src = (b0 ^ 1) * 32
                            nc.scalar.dma_start(
                                out=rot[b0 * 32:(b0 + 1) * 32, :],
                                in_=qraw[src:src + 32, :])
                        nc.vector.tensor_tensor(
                            qkT[jt][:, ss], qkps[:], cosT[:, ss], ALU.mult)
                        rotm = s512.tile([128, 512], F32, tag="rotm", name="rotm")
                        nc.vector.tensor_tensor(
                            rotm[:], rot[:], s2T[:, ss], ALU.mult)
                        nc.gpsimd.tensor_tensor(
                            qkT[jt][:, ss], qkT[jt][:, ss], rotm[:], ALU.add)

            # ---------------- attention for one head pair ----------------
            def attn_chunk(p, t):
                qT, kT = qkT[2 * p], qkT[2 * p + 1]
                if True:
                    pav = [gpps.tile([128, 512], F32, tag="ps1", name="pav")
                           for _ in range(2)]
                    njp = 2 * t + 2
                    for jp in range(njp):
                        spt = [sps.tile([128, 2, 512], F32, tag="sps",
                                        name="spt") for _ in range(2)]
                        for jj in range(2):          # A,B interleaved: row-
                            for hh in range(2):      # group concurrency on PE
                                hb = 64 * hh
                                jt = 2 * jp + jj
                                nc.tensor.matmul(
                                    spt[hh][:, jj, :],
                                    kT[hb:hb + 64, jt * 128:(jt + 1) * 128],
                                    qT[hb:hb + 64, t * 512:(t + 1) * 512],
                                    start=True, stop=True)
                        eT = [None, None]
                        d0 = 2 * jp - 4 * t
                        for hh in range(2):
                            et = expp.tile([128, 2, 512], F16, tag="expp",
                                           name="et")
                            if d0 < 0:
                                nc.scalar.activation(et[:], spt[hh][:], EXP,
                                                     scale=scale)
                            else:
                                for jj in range(2):
                                    d = d0 + jj
                                    if d * 128 > 0:
                                        nc.vector.memset(
                                            et[:, jj, 0:d * 128], 0.0)
                                    nc.scalar.activation(
                                        et[:, jj, d * 128:512],
                                        spt[hh][:, jj, d * 128:512], EXP,
                                        scale=scale)
                                    nc.vector.tensor_tensor(
                                        et[:, jj, d * 128:(d + 1) * 128],
                                        et[:, jj, d * 128:(d + 1) * 128],
                                        tri[:], ALU.mult)
                            eT[hh] = et
                        for hh in range(2):
                            h = 2 * p + hh
                            for jj in range(2):
                                jt = 2 * jp + jj
                                nc.tensor.matmul(
                                    pav[hh][0:65, :],
                                    v_sb[:, jt, h, 0:65],
                                    eT[hh][:, jj, :],
                                    start=(jp == 0 and jj == 0),
                                    stop=(jp == njp - 1 and jj == 1))
                    for hh in range(2):
                        oslc = outT[p][64 * hh:64 * hh + 64,
                                       t * 512:(t + 1) * 512]
                        oraw = normp.tile([64, 512], F32, tag="oraw", name="oraw")
                        nc.vector.tensor_copy(oraw[:], pav[hh][0:64, :])
                        srow = normp.tile([1, 512], F32, tag="srow", name="srow")
                        nc.vector.tensor_copy(srow[:], pav[hh][64:65, :])
                        rstage = normp.tile([1, 512], F32, tag="rst", name="rst")
                        scr = normp.tile([1, 512], F32, tag="scr", name="scr")
                        nc.vector.reciprocal_approx_accurate(
                            out=rstage[:], in_=srow[:], scratch=scr[:])
                        brec = normp.tile([64, 512], F32, tag="brec", name="brec")
                        nc.gpsimd.partition_broadcast(brec[:], rstage[:])
                        nc.vector.tensor_tensor(
                            oslc, oraw[:], brec[:], ALU.mult)

            def qk_half(jh):
                wqk = qk_load_w(jh)
                for jlpair in ([0, 1], [2, 3]):
                    for sc in range(SC):
                        xq = load_x_chunk(sc)
                        for jl in jlpair:
                            qk_group(jh, wqk, jl, sc, xq)

            qk_half(0)
            for t in range(IC):
                attn_chunk(0, t)
            for t in range(IC):
                attn_chunk(1, t)
            qk_half(1)
            for t in range(IC):
                attn_chunk(2, t)
            for t in range(IC):
                attn_chunk(3, t)

            # ---------------- output projection ----------------
            wout = w16.tile([128, NP, D // 512, 512], F16, tag="w16",
                            name="wout")
            for p in range(NP):
                for dc in range(D // 512):
                    nc.sync.dma_start(
                        out=wout[:, p, dc, :],
                        in_=Wout_d[p * 128:(p + 1) * 128,
                                   dc * 512:(dc + 1) * 512])
            NDC = D // 512
            for st in range(ST):
                yp2 = [gpps.tile([128, 512], F32, tag="ps1", name="yps")
                       for _ in range(NDC)]
                for p in range(NP):
                    for dc in range(NDC):
                        nc.tensor.matmul(
                            yp2[dc][:], outT[p][:, st * 128:(st + 1) * 128],
                            wout[:, p, dc, :],
                            start=(p == 0), stop=(p == NP - 1))
                for dc in range(NDC):
                    yst = s512.tile([128, 512], F32, tag="s512", name="yst")
                    nc.vector.tensor_copy(yst[:], yp2[dc][:])
                    nc.sync.dma_start(
                        out=y_d[st * 128:(st + 1) * 128,
                                dc * 512:(dc + 1) * 512],
                        in_=yst[:])
    nc.compile()
    return nc


def make_tables(S=2048, hd=64):
    inv_freq = 1.0 / (ROPE_BASE ** (np.arange(0, hd, 2, dtype=np.float64) / hd))
    t = np.arange(S, dtype=np.float64)
    freqs = np.outer(t, inv_freq)                    # [S, 32]
    emb = np.concatenate([freqs, freqs], axis=-1)    # [S, 64]
    cos1 = np.cos(emb).T.astype(np.float32)          # [64, S]
    sin1 = np.sin(emb).T.astype(np.float32)
    s2_1 = sin1.copy()
    s2_1[0:32] = -s2_1[0:32]
    cosT = np.concatenate([cos1, cos1], axis=0).astype(np.float16)  # [128, S]
    s2T = np.concatenate([s2_1, s2_1], axis=0).astype(np.float16)
    tri = np.tril(np.ones((128, 128), np.float32)).T.astype(np.float16)
    # tri[j, i] = 1 iff j <= i  (lower-tri transposed = upper-tri in [j, i])
    return cosT, s2T, tri


def make_core_inputs(x, Wqkv, Wout, b, g, HL=8, hd=64):
    """Host-side shard prep for core (batch b, head group g)."""
    B, S, D = x.shape
    H = D // hd
    heads = list(range(g * HL, (g + 1) * HL))
    Wq = Wqkv[:, 0:D].reshape(D, H, hd)
    Wk = Wqkv[:, D:2 * D].reshape(D, H, hd)
    Wv = Wqkv[:, 2 * D:3 * D].reshape(D, H, hd)
    # Wqk j-tile order: q(h0,h1), k(h0,h1), q(h2,h3), k(h2,h3), ...
    blocks = []
    for p in range(HL // 2):
        h0, h1 = heads[2 * p], heads[2 * p + 1]
        blocks.append(np.concatenate([Wq[:, h0], Wq[:, h1]], axis=1))
        blocks.append(np.concatenate([Wk[:, h0], Wk[:, h1]], axis=1))
    Wqk_host = np.ascontiguousarray(np.concatenate(blocks, axis=1), np.float16)
    Wv_host = np.ascontiguousarray(
        Wv[:, heads].reshape(D, HL * hd), np.float16)
    Wout_host = np.ascontiguousarray(
        Wout[g * HL * hd:(g + 1) * HL * hd, :], np.float16)
    xT = np.ascontiguousarray(x[b].T, np.float16)
    cosT, s2T, tri = make_tables(S, hd)
    return {"xT": xT, "Wqk": Wqk_host, "Wv": Wv_host, "Wout": Wout_host,
            "cosT": cosT, "S2T": s2T, "tri": tri}


_NC_CACHE = {}
TRACE = False          # test-only: capture NTFF profile + exec time
LAST_EXEC_NS = None
LAST_RESULT = None


def _enable_ntff_hook():
    import types
    import trn_agent_boot.trn_boot as tb
    import concourse.bass_utils as bu
    m = types.ModuleType("antenv.axon_hooks")
    _hook = [None]
    m.set_axon_ntff_profile_hook = lambda h: _hook.__setitem__(0, h)
    m.get_axon_ntff_profile_hook = lambda: _hook[0]
    sys.modules["antenv.axon_hooks"] = m
    m.set_axon_ntff_profile_hook(
        tb._ntff_profile_via_ctypes("/opt/axon/libaxon_pjrt.so"))
    bu.upload_artifacts = lambda tmpdir: ""


def kernel(x, Wqkv, Wout):
    global LAST_EXEC_NS, LAST_RESULT
    B, S, D = x.shape
    key = (B, S, D)
    if key not in _NC_CACHE:
        _NC_CACHE[key] = build_core_program(S=S, D=D)
    nc = _NC_CACHE[key]
    in_maps = []
    for core in range(8):
        b, g = core // 2, core % 2
        in_maps.append(make_core_inputs(np.asarray(x), np.asarray(Wqkv),
                                        np.asarray(Wout), b, g))
    kw = {}
    if TRACE:
        _enable_ntff_hook()
        kw = dict(trace=True, trace_cores=[0])
    res = run_bass_kernel_spmd(nc, in_maps, core_ids=list(range(8)), **kw)
    LAST_EXEC_NS = res.exec_time_ns
    LAST_RESULT = res
    y = np.empty((B, S, D), np.float32)
    for b in range(B):
        y[b] = res.results[2 * b]["y"] + res.results[2 * b + 1]["y"]
    return y



# revision 3
# speedup vs baseline: 1.0386x; 1.0386x over previous
"""Causal self-attention with RoPE on 8 Trainium2 NeuronCores.

Sharding: batch (4) x head-group (2 groups of 8 heads) -> 8 cores.
Each core computes, for its (batch b, head group g):
  qkv projection (fp16 matmuls, x + weights SBUF-resident, loaded once),
  RoPE (rotate-half via +-1 permutation matmul on PE + fp16 DVE mults),
  causal attention (scores fp16, softmax exp on ACT with folded 1/8 scale
  in full-width calls + post-masking, probs/V in fp16, exact-sum
  normalization via ones-column),
  output projection partial (fp16) interleaved into the attention phase.
Host sums the two head-group partials per batch.
"""
import sys

sys.path.insert(0, "/opt/trn_rl_repo")

import numpy as np

import concourse.bass as bass  # noqa: F401
import concourse.mybir as mybir
import concourse.tile as tile
from concourse import bacc
from concourse.bass_utils import run_bass_kernel_spmd

dt = mybir.dt
F32, F16 = dt.float32, dt.float16
ALU = mybir.AluOpType
EXP = mybir.ActivationFunctionType.Exp

ROPE_BASE = 10000.0


def build_core_program(S=2048, D=1024, HL=8, hd=64):
    """Bass program for one core.

    Emission order: loads; v-proj; qk-proj all 8 j-tiles (with RoPE);
    attention pairs 0,1; pairs 2,3 with out-proj tiles interleaved at
    t-chunk granularity so the PE stays fed while ACT runs exp.
    """
    assert hd == 64
    NP = HL // 2           # 4 head pairs
    DT = D // 128          # 8 contraction tiles
    SC = S // 512          # 4 sequence chunks
    ST = S // 128          # 16 seq tiles
    NJT = 2 * NP           # 8 qk j-tiles
    IC = S // 512          # 4 attention query chunks
    NDC = D // 512         # 2 out-proj column chunks
    scale = hd ** -0.5

    nc = bacc.Bacc("TRN2", target_bir_lowering=False, debug=False)
    xT_d = nc.dram_tensor("xT", [D, S], F16, kind="ExternalInput").ap()
    Wqk_d = nc.dram_tensor("Wqk", [D, NJT * 128], F16, kind="ExternalInput").ap()
    Wv_d = nc.dram_tensor("Wv", [D, HL * hd], F16, kind="ExternalInput").ap()
    Wout_d = nc.dram_tensor("Wout", [HL * hd, D], F16, kind="ExternalInput").ap()
    cos_d = nc.dram_tensor("cosT", [128, S], F16, kind="ExternalInput").ap()
    sin_d = nc.dram_tensor("sinT", [128, S], F16, kind="ExternalInput").ap()
    tri_d = nc.dram_tensor("tri", [128, 128], F16, kind="ExternalInput").ap()
    rotP_d = nc.dram_tensor("rotP", [128, 128], F16, kind="ExternalInput").ap()
    y_d = nc.dram_tensor("y", [S, D], F32, kind="ExternalOutput").ap()

    with tile.TileContext(nc) as tc:
        with tc.tile_pool(name="persist", bufs=1) as pp, \
             tc.tile_pool(name="q16p", bufs=4) as q16p, \
             tc.tile_pool(name="expp", bufs=6) as expp, \
             tc.tile_pool(name="normp", bufs=4) as normp, \
             tc.tile_pool(name="ystp", bufs=2) as ystp, \
             tc.tile_pool(name="projps", bufs=2, space="PSUM") as projps, \
             tc.tile_pool(name="sps", bufs=2, space="PSUM") as sps, \
             tc.tile_pool(name="pavp", bufs=2, space="PSUM") as pavp:

            # ---------------- persistent SBUF tensors ----------------
            xq = pp.tile([128, DT, S], F16, tag="xq")
            wv = pp.tile([128, DT, HL * hd], F16, tag="wv")
            wqk = pp.tile([128, DT, NJT * 128], F16, tag="wqk")
            wout = pp.tile([128, NP, NDC, 512], F16, tag="wout")
            qkT = [pp.tile([128, S], F16, tag=f"qkT{j}", name=f"qkT{j}")
                   for j in range(NJT)]
            v_sb = pp.tile([128, ST, HL, 66], F16, tag="v_sb")
            outT = [pp.tile([128, S], F16, tag=f"outT{p}", name=f"outT{p}")
                    for p in range(NP)]
            cosT = pp.tile([128, S], F16, tag="cosT")
            sinT = pp.tile([128, S], F16, tag="sinT")
            tri = pp.tile([128, 128], F16, tag="tri")
            rotP = pp.tile([128, 128], F16, tag="rotP")

            for ddt in range(DT):
                nc.sync.dma_start(
                    out=wv[:, ddt, :], in_=Wv_d[ddt * 128:(ddt + 1) * 128, :])
            for ddt in range(DT):
                nc.sync.dma_start(
                    out=xq[:, ddt, :], in_=xT_d[ddt * 128:(ddt + 1) * 128, :])
            for ddt in range(DT):
                nc.sync.dma_start(
                    out=wqk[:, ddt, :], in_=Wqk_d[ddt * 128:(ddt + 1) * 128, :])
            nc.sync.dma_start(out=cosT[:], in_=cos_d[:])
            nc.sync.dma_start(out=sinT[:], in_=sin_d[:])
            nc.sync.dma_start(out=tri[:], in_=tri_d[:])
            nc.sync.dma_start(out=rotP[:], in_=rotP_d[:])
            for p in range(NP):
                for dc in range(NDC):
                    nc.sync.dma_start(
                        out=wout[:, p, dc, :],
                        in_=Wout_d[p * 128:(p + 1) * 128,
                                   dc * 512:(dc + 1) * 512])
            nc.vector.memset(v_sb[:, :, :, 64:65], 1.0)

            # ---------------- V projection (copies on ACT) ----------------
            for st in range(ST):
                vps = projps.tile([128, 512], F32, tag="pjps", name="vps")
                for ddt in range(DT):
                    nc.tensor.matmul(
                        vps[:],
                        xq[:, ddt, st * 128:(st + 1) * 128],
                        wv[:, ddt, :], start=(ddt == 0), stop=(ddt == DT - 1))
                nc.scalar.copy(
                    v_sb[:, st, :, 0:64],
                    vps[:].rearrange("p (h c) -> p h c", h=HL))

            # ---------------- QK projection + RoPE (one group) ----------------
            def qk_group(jt, sc):
                ss = slice(sc * 512, (sc + 1) * 512)
                qkps = projps.tile([128, 512], F32, tag="pjps", name="qkps")
                for ddt in range(DT):
                    nc.tensor.matmul(
                        qkps[:], wqk[:, ddt, jt * 128:(jt + 1) * 128],
                        xq[:, ddt, ss],
                        start=(ddt == 0), stop=(ddt == DT - 1))
                q16 = q16p.tile([128, 512], F16, tag="q16", name="q16")
                nc.scalar.copy(q16[:], qkps[:])
                rot_ps = projps.tile([128, 512], F32, tag="pjps", name="rotps")
                nc.tensor.matmul(rot_ps[:], rotP[:], q16[:],
                                 start=True, stop=True)
                rotm = q16p.tile([128, 512], F16, tag="q16", name="rotm")
                nc.vector.tensor_tensor(
                    rotm[:], rot_ps[:], sinT[:, ss], ALU.mult)
                nc.vector.tensor_tensor(
                    qkT[jt][:, ss], q16[:], cosT[:, ss], ALU.mult)
                nc.vector.tensor_tensor(
                    qkT[jt][:, ss], qkT[jt][:, ss], rotm[:], ALU.add)

            for jt in range(NJT):
                for sc in range(SC):
                    qk_group(jt, sc)

            # ---------------- attention for one head pair ----------------
            def attn_chunk(p, t):
                qT, kT = qkT[2 * p], qkT[2 * p + 1]
                pav = [pavp.tile([128, 512], F32, tag="pav", name="pav")
                       for _ in range(2)]
                njp = 2 * t + 2
                for jp in range(njp):
                    d0 = 2 * jp - 4 * t
                    for hh in range(2):
                        hb = 64 * hh
                        spt = sps.tile([128, 2, 512], F32, tag="spt",
                                       name="spt")
                        for jj in range(2):
                            jt = 2 * jp + jj
                            nc.tensor.matmul(
                                spt[:, jj, :],
                                kT[hb:hb + 64, jt * 128:(jt + 1) * 128],
                                qT[hb:hb + 64, t * 512:(t + 1) * 512],
                                start=True, stop=True)
                        et = expp.tile([128, 2, 512], F16, tag="expp",
                                       name="et")
                        nc.scalar.activation(et[:], spt[:], EXP, scale=scale)
                        if d0 >= 0:
                            for jj in range(2):
                                d = d0 + jj
                                if d * 128 > 0:
                                    nc.vector.memset(
                                        et[:, jj, 0:d * 128], 0.0)
                                nc.gpsimd.tensor_tensor(
                                    et[:, jj, d * 128:(d + 1) * 128],
                                    et[:, jj, d * 128:(d + 1) * 128],
                                    tri[:], ALU.mult)
                        h = 2 * p + hh
                        for jj in range(2):
                            jt = 2 * jp + jj
                            nc.tensor.matmul(
                                pav[hh][0:65, :],
                                v_sb[:, jt, h, 0:65],
                                et[:, jj, :],
                                start=(jp == 0 and jj == 0),
                                stop=(jp == njp - 1 and jj == 1))
                for hh in range(2):
                    oslc = outT[p][64 * hh:64 * hh + 64,
                                   t * 512:(t + 1) * 512]
                    oraw = normp.tile([64, 512], F32, tag="oraw", name="oraw")
                    nc.vector.tensor_copy(oraw[:], pav[hh][0:64, :])
                    srow = normp.tile([1, 512], F32, tag="srow", name="srow")
                    nc.vector.tensor_copy(srow[:], pav[hh][64:65, :])
                    rstage = normp.tile([1, 512], F32, tag="rst", name="rst")
                    scr = normp.tile([1, 512], F32, tag="scr", name="scr")
                    nc.vector.reciprocal_approx_accurate(
                        out=rstage[:], in_=srow[:], scratch=scr[:])
                    brec = normp.tile([64, 512], F32, tag="brec", name="brec")
                    nc.gpsimd.partition_broadcast(brec[:], rstage[:])
                    nc.vector.tensor_tensor(
                        oslc, oraw[:], brec[:], ALU.mult)

            # ---------------- out-proj for one 512-chunk of seq ----------------
            def out_chunk(t):
                for st in range(4 * t, 4 * t + 4):
                    yp2 = [projps.tile([128, 512], F32, tag="pjps",
                                       name="yps") for _ in range(NDC)]
                    for pb in range(NP):
                        for dc in range(NDC):
                            nc.tensor.matmul(
                                yp2[dc][:],
                                outT[pb][:, st * 128:(st + 1) * 128],
                                wout[:, pb, dc, :],
                                start=(pb == 0), stop=(pb == NP - 1))
                    yst = ystp.tile([128, NDC * 512], F32, tag="yst",
                                    name="yst")
                    for dc in range(NDC):
                        nc.vector.tensor_copy(
                            yst[:, dc * 512:(dc + 1) * 512], yp2[dc][:])
                    nc.sync.dma_start(
                        out=y_d[st * 128:(st + 1) * 128, :], in_=yst[:])

            for t in range(IC):
                attn_chunk(0, t)
            for t in range(IC):
                attn_chunk(1, t)
            for t in range(IC):
                attn_chunk(2, t)
                attn_chunk(3, t)
                out_chunk(t)
    nc.compile()
    return nc


def make_tables(S=2048, hd=64):
    inv_freq = 1.0 / (ROPE_BASE ** (np.arange(0, hd, 2, dtype=np.float64) / hd))
    t = np.arange(S, dtype=np.float64)
    freqs = np.outer(t, inv_freq)                    # [S, 32]
    emb = np.concatenate([freqs, freqs], axis=-1)    # [S, 64]
    cos1 = np.cos(emb).T.astype(np.float32)          # [64, S]
    sin1 = np.sin(emb).T.astype(np.float32)
    cosT = np.concatenate([cos1, cos1], axis=0).astype(np.float16)  # [128, S]
    sinT = np.concatenate([sin1, sin1], axis=0).astype(np.float16)
    tri = np.tril(np.ones((128, 128), np.float32)).T.astype(np.float16)
    # tri[j, i] = 1 iff j <= i  (lower-tri transposed = upper-tri in [j, i])
    # rotP.T @ q = rotate_half(q) with the sign folded in, per 64-dim head:
    #   out[j] = -q[j+32] for j%64 in [0,32), out[j] = q[j-32] for [32,64)
    rotP = np.zeros((128, 128), np.float16)
    for j in range(128):
        base = (j // 64) * 64
        jj = j % 64
        if jj < 32:
            rotP[base + jj + 32, j] = -1.0
        else:
            rotP[base + jj - 32, j] = 1.0
    return cosT, sinT, tri, rotP


def make_core_inputs(x, Wqkv, Wout, b, g, HL=8, hd=64):
    """Host-side shard prep for core (batch b, head group g)."""
    B, S, D = x.shape
    H = D // hd
    heads = list(range(g * HL, (g + 1) * HL))
    Wq = Wqkv[:, 0:D].reshape(D, H, hd)
    Wk = Wqkv[:, D:2 * D].reshape(D, H, hd)
    Wv = Wqkv[:, 2 * D:3 * D].reshape(D, H, hd)
    # Wqk j-tile order: q(h0,h1), k(h0,h1), q(h2,h3), k(h2,h3), ...
    blocks = []
    for p in range(HL // 2):
        h0, h1 = heads[2 * p], heads[2 * p + 1]
        blocks.append(np.concatenate([Wq[:, h0], Wq[:, h1]], axis=1))
        blocks.append(np.concatenate([Wk[:, h0], Wk[:, h1]], axis=1))
    Wqk_host = np.ascontiguousarray(np.concatenate(blocks, axis=1), np.float16)
    Wv_host = np.ascontiguousarray(
        Wv[:, heads].reshape(D, HL * hd), np.float16)
    Wout_host = np.ascontiguousarray(
        Wout[g * HL * hd:(g + 1) * HL * hd, :], np.float16)
    xT = np.ascontiguousarray(x[b].T, np.float16)
    cosT, sinT, tri, rotP = make_tables(S, hd)
    return {"xT": xT, "Wqk": Wqk_host, "Wv": Wv_host, "Wout": Wout_host,
            "cosT": cosT, "sinT": sinT, "tri": tri, "rotP": rotP}


_NC_CACHE = {}
TRACE = False          # test-only: capture NTFF profile + exec time
LAST_EXEC_NS = None
LAST_RESULT = None


def _enable_ntff_hook():
    import types
    import trn_agent_boot.trn_boot as tb
    import concourse.bass_utils as bu
    m = types.ModuleType("antenv.axon_hooks")
    _hook = [None]
    m.set_axon_ntff_profile_hook = lambda h: _hook.__setitem__(0, h)
    m.get_axon_ntff_profile_hook = lambda: _hook[0]
    sys.modules["antenv.axon_hooks"] = m
    m.set_axon_ntff_profile_hook(
        tb._ntff_profile_via_ctypes("/opt/axon/libaxon_pjrt.so"))
    bu.upload_artifacts = lambda tmpdir: ""


def kernel(x, Wqkv, Wout):
    global LAST_EXEC_NS, LAST_RESULT
    B, S, D = x.shape
    key = (B, S, D)
    if key not in _NC_CACHE:
        _NC_CACHE[key] = build_core_program(S=S, D=D)
    nc = _NC_CACHE[key]
    in_maps = []
    for core in range(8):
        b, g = core // 2, core % 2
        in_maps.append(make_core_inputs(np.asarray(x), np.asarray(Wqkv),
                                        np.asarray(Wout), b, g))
    kw = {}
    if TRACE:
        _enable_ntff_hook()
        kw = dict(trace=True, trace_cores=[0])
    res = run_bass_kernel_spmd(nc, in_maps, core_ids=list(range(8)), **kw)
    LAST_EXEC_NS = res.exec_time_ns
    LAST_RESULT = res
    y = np.empty((B, S, D), np.float32)
    for b in range(B):
        y[b] = res.results[2 * b]["y"] + res.results[2 * b + 1]["y"]
    return y
